# revision 54
# baseline (speedup 1.0000x reference)
"""Multi-head attention (B=2, S=2048, D=1024, H=16) on 8 NeuronCores.

Sharding: core = (batch b, head-group hg), 4 heads / 256 hidden dims per
core; host sums the 4 head-group partials per batch and adds
bo_aug = bo + bv @ Wo.

v1 design (vs the previous all-bf16 kernel):
  * Q^T/K^T are stored fp8(e4m3) in a d-half-grouped layout
    [128 part = 4 heads x 32 d, 2 group = d-half, S] so each scores chunk
    [128 kpos, 512 q] is ONE DoubleRow fp8 matmul (256 PE cycles instead of
    512): scores PE cost halves to 27us.  The grouped layout falls out of
    the K/Q projection for free: the projection m-tiles are d-halves
    (all 4 heads x d mod 32) instead of head-pairs, and the PSUM->SBUF
    bias-copy writes the fp8 group slice directly (partition-preserving).
    Accuracy: q/k fp8 rounding costs ~1.2e-2 rel err (gate 2e-2); numpy-
    validated.  Everything else stays bf16 (x/V/ctx/out fp8 all fail the
    gate, and PE is no longer the bottleneck).
  * ACT exp is the bottleneck (109us floor + 185ns/instruction): scores
    PSUM alternates a 2-bank [128,1024] and a 3-bank [128,1536] tile so
    exps average 0.95ns/col instead of 1.01 (ACT busy 128us vs 133us).
    PSUM: scA(2) + scB(3) + cx(2) + mm(1) = 8 banks.
  * ACT does exp only (plus 2 startup q-biases + drain-tail copies);
    biases/norm/copies live on DVE.
  * PE warms up on junk matmuls from t=0 so the p-state ramp (full clock
    only after 3us continuously busy) is spent before the first real mm.
  * xT streams in 512-column quarters so the n0 K/Q rounds start ~2us in;
    startup K/Q rounds interleave per k-tile chasing the DMA.

Per-core dataflow (f32 PSUM accumulation everywhere):
  K^T/Q^T rounds: per (g=d-half, n=512 q cols): 8 bf16 matmuls contract
    x's 1024 dims -> PSUM [128, 512]; DVE tensor_scalar adds the bias and
    writes fp8 into kt8/qt8[:, g, n-cols].
  scores^T chunk (u=(n,p), j, hh, tt): DoubleRow mm, kpos tile t=2j+tt,
    head h=2p+hh -> sc tile slice [128, 512].
  exp on ACT over whole sc tiles (1024/1536 cols) -> at tiles bf16.
  ctx: per (u, j): 16 bf16 mms (attn chunk slices vs vaug[t]) accumulate
    [128, 4qt x (1 ones + 64 v)] per head in cx banks.
  norm: DVE reciprocal + per-qt tensor_scalar -> ctx_n bf16 [q, 128 c].
  transpose ctx_n via PE (4 per unit into one borrowed cx bank), DVE
    copies -> ctxT [128 c, 512 q].
  out: per (s, dh): 2-step bf16 mm over ctxT p-halves vs wo -> [128, 512],
    copy to SBUF (DVE mid-stream, ACT+DVE in the drain tail), DMA out.

Host-side input layouts:
  xT  [1024, 2048]  x[b].T                                        bf16
  wq/wk [2, 128, 8*128]  g = d-half split, k-tile-major columns,
        column j of half g = W[:, (j//32)*64 + 32*g + (j%32)]      bf16
  bq/bk [128, 2]    bias columns matching wq/wk halves             f32
  wv  [128, 8*256]  k-tile-major columns (natural c order)         bf16
  wo  [128, 2*1024] head-pair-stacked rows side by side            bf16
"""

from contextlib import ExitStack

import ml_dtypes
import numpy as np

import concourse.bass as bass
import concourse.mybir as mybir
import concourse.tile as tile
from concourse import bacc
from concourse.bass import ts
from concourse import bass_utils
from concourse.masks import make_identity

S = 2048
D = 1024
H = 16
HD = 64
HPC = 4          # heads per core
C = HPC * HD     # 256 hidden dims per core
N_CORES = 8

BF16 = mybir.dt.bfloat16
F32 = mybir.dt.float32
FP8 = mybir.dt.float8e4
NP_BF16 = ml_dtypes.bfloat16
NP_FP8 = ml_dtypes.float8_e4m3

_CACHE = {}


def _build_nc():
    nc = bacc.Bacc(
        "TRN2", target_bir_lowering=False, debug=False, num_devices=N_CORES
    )

    xT = nc.dram_tensor("xT", [D, S], BF16, kind="ExternalInput").ap()
    wq = nc.dram_tensor("wq", [2, 128, 8 * 128], BF16, kind="ExternalInput").ap()
    wk = nc.dram_tensor("wk", [2, 128, 8 * 128], BF16, kind="ExternalInput").ap()
    wv = nc.dram_tensor("wv", [128, 8 * C], BF16, kind="ExternalInput").ap()
    wo = nc.dram_tensor("wo", [128, 2 * D], BF16, kind="ExternalInput").ap()
    bq = nc.dram_tensor("bq", [128, 2], F32, kind="ExternalInput").ap()
    bk = nc.dram_tensor("bk", [128, 2], F32, kind="ExternalInput").ap()
    out = nc.dram_tensor("out", [S, D], BF16, kind="ExternalOutput").ap()

    with tile.TileContext(nc, pool_alloc_mode="queue") as tc, ExitStack() as ctx:
        ep = ctx.enter_context

        xt_pool = ep(tc.tile_pool(name="xt", bufs=1))
        w_pool = ep(tc.tile_pool(name="w", bufs=4))
        wv_pool = ep(tc.tile_pool(name="wv", bufs=1))
        wo_pool = ep(tc.tile_pool(name="wo", bufs=1))
        small_pool = ep(tc.tile_pool(name="small", bufs=4))
        const_pool = ep(tc.tile_pool(name="const", bufs=2))
        qk8_pool = ep(tc.tile_pool(name="qk8", bufs=2))
        atA_pool = ep(tc.tile_pool(name="atA", bufs=18))
        atB_pool = ep(tc.tile_pool(name="atB", bufs=18))
        vaug_pool = ep(tc.tile_pool(name="vaug", bufs=16))
        recip_pool = ep(tc.tile_pool(name="recip", bufs=8))
        ctxn_pool = ep(tc.tile_pool(name="ctxn", bufs=32))
        ctxT_pool = ep(tc.tile_pool(name="ctxT", bufs=8))
        outsb_pool = ep(tc.tile_pool(name="outsb", bufs=8))
        scA_ps = ep(tc.tile_pool(name="scA", bufs=1, space="PSUM"))
        scB_ps = ep(tc.tile_pool(name="scB", bufs=1, space="PSUM"))
        cx_ps = ep(tc.tile_pool(name="cx", bufs=2, space="PSUM"))
        mm_ps = ep(tc.tile_pool(name="mm", bufs=1, space="PSUM"))

        # ---- t=0: constants + PE warmup (pstate ramp burns out on junk) ----
        junk = const_pool.tile([128, 512], BF16, tag="junk", name="junk")
        nc.gpsimd.memset(junk[:], 0.0)
        junk2 = const_pool.tile([128, 32], BF16, tag="junk2", name="junk2")
        # dummy exp: pulls the 1.28us activation-table load to t~0 while the
        # ACT engine is idle instead of right before the first real exp
        nc.scalar.activation(junk2[:], junk[:, 0:32],
                             mybir.ActivationFunctionType.Exp, scale=0.125)
        ident = const_pool.tile([128, 128], BF16, tag="id", name="ident")
        make_identity(nc, ident[:])

        warm = scA_ps.tile([128, 1024], F32, tag="scA", name="warm")
        for i in range(11):
            nc.tensor.matmul(
                warm[:, ts(i % 2, 512)], lhsT=junk[:, 0:128], rhs=junk[:],
                start=True, stop=True, skip_group_check=True,
            )

        # ---- DMAs ----
        # HWDGE serializes DMA *issues* at ~0.65us each, the shared data
        # engines serialize transfers (~2.9us/MB): few DMAs, need-ordered.
        # Order: wk, wq (one DMA each), xT quarter 0 (8), bk, bq, xT
        # quarter 1 (8), wv, xT halves 2+3 (8), wo.
        wk_sb = w_pool.tile([128, 2 * 1024], BF16, tag="w", name="wk_sb")
        nc.sync.dma_start(
            wk_sb[:].rearrange("p (g c) -> p g c", g=2),
            wk.rearrange("g p c -> p g c"),
        )
        wq_sb = w_pool.tile([128, 2 * 1024], BF16, tag="w", name="wq_sb")
        nc.sync.dma_start(
            wq_sb[:].rearrange("p (g c) -> p g c", g=2),
            wq.rearrange("g p c -> p g c"),
        )

        bk_sb = small_pool.tile([128, 2], F32, tag="b", name="bk_sb")
        nc.sync.dma_start(bk_sb[:], bk[:])
        bq_sb = small_pool.tile([128, 2], F32, tag="b", name="bq_sb")
        nc.sync.dma_start(bq_sb[:], bq[:])

        # xT lives in ONE [128, 8 k-tiles x 2048] tile so a whole column
        # range of all 8 k-tiles loads as a single DMA (HWDGE issues are the
        # scarce resource at ~0.65us each).  Quarter 0 still goes per-tile so
        # the startup K/Q matmuls can chase the individual tile sems.
        xtbig = xt_pool.tile([128, 8 * S], BF16, tag="xt", name="xtbig")
        xt = [xtbig[:, 2048 * k : 2048 * (k + 1)] for k in range(8)]

        for k in range(8):
            nc.sync.dma_start(xt[k][:, 0:512], xT[ts(k, 128), 0:512])
        def load_xt_range(lo, width):
            nc.sync.dma_start(
                xtbig[:].rearrange("p (k c) -> p k c", k=8)[:, :, lo : lo + width],
                xT.rearrange("(k p) c -> p k c", k=8)[:, :, lo : lo + width],
            )

        load_xt_range(512, 512)     # quarter 1, one DMA
        load_xt_range(1024, 512)    # quarter 2, one DMA
        load_xt_range(1536, 512)    # quarter 3, one DMA
        wv_sb = wv_pool.tile([128, 8 * C], BF16, tag="wv", name="wv_sb")
        nc.sync.dma_start(wv_sb[:], wv[:])
        wo_sb = wo_pool.tile([128, 2 * D], BF16, tag="wo", name="wo_sb")
        nc.sync.dma_start(wo_sb[:], wo[:])

        # DMA completion estimates (ns) for scheduler gating
        t_xq = [8600.0, 10600.0, 12100.0, 13500.0]
        t_wv = 14300.0

        # ---- fp8 K^T/Q^T tiles [128, 2 group, 2048] ----
        qt8 = qk8_pool.tile([128, 2 * S], FP8, tag="qk8", name="qt8")
        kt8 = qk8_pool.tile([128, 2 * S], FP8, tag="qk8", name="kt8")
        qt8r = qt8[:].rearrange("p (g s) -> p g s", g=2)
        kt8r = kt8[:].rearrange("p (g s) -> p g s", g=2)

        # ---- K/Q projection rounds ----
        kq_done = {}

        def kq_dst(which, g, n):
            r = qt8r if which == "q" else kt8r
            return r[:, g, ts(n, 512)]

        def kq_bias(which, g):
            b = bq_sb if which == "q" else bk_sb
            return b[:, g : g + 1]

        def kq_w(which, g):
            w = wq_sb if which == "q" else wk_sb
            return w[:, 1024 * g : 1024 * (g + 1)]

        def emit_kq_round(which, g, n, pool, tag, bias_act=False):
            ps = pool.tile([128, 512], F32, tag=tag, name=f"ps{which}{g}{n}")
            for k in range(8):
                nc.tensor.matmul(
                    ps[:],
                    lhsT=kq_w(which, g)[:, ts(k, 128)],
                    rhs=xt[k][:, 512 * n : 512 * n + 512],
                    start=(k == 0),
                    stop=(k == 7),
                )
            if bias_act:
                nc.scalar.activation(
                    kq_dst(which, g, n), ps[:],
                    mybir.ActivationFunctionType.Identity,
                    bias=kq_bias(which, g),
                )
            else:
                nc.vector.tensor_scalar(
                    kq_dst(which, g, n), ps[:], kq_bias(which, g), None,
                    mybir.AluOpType.add,
                )
            kq_done[(which, g, n)] = True

        def emit_kq_startup():
            """All four n0 rounds interleaved per k-tile so each round's
            matmuls chase the quarter-0 DMAs instead of serializing after
            them.  q biases on the still-idle ACT engine, k on DVE."""
            rounds = [("k", 0), ("q", 0), ("k", 1), ("q", 1)]
            pools = [(cx_ps, "cx", 512), (cx_ps, "cx", 512),
                     (mm_ps, "mm", 512), (scB_ps, "scB", 1536)]
            pss = {}
            for (which, g), (pool, tag, w) in zip(rounds, pools):
                pss[(which, g)] = pool.tile([128, w], F32, tag=tag,
                                            name=f"ps{which}{g}0")
            # the g0 pair first (2 mms per arriving k-tile matches the DMA
            # rate), then the g1 pair on resident tiles; each pair's biases
            # fire as soon as the pair completes (q on ACT, k on DVE)
            for gpair in range(2):
                for k in range(8):
                    for which in ("k", "q"):
                        nc.tensor.matmul(
                            pss[(which, gpair)][:, 0:512],
                            lhsT=kq_w(which, gpair)[:, ts(k, 128)],
                            rhs=xt[k][:, 0:512],
                            start=(k == 0),
                            stop=(k == 7),
                            skip_group_check=True,
                        )
                nc.scalar.activation(
                    kq_dst("q", gpair, 0), pss[("q", gpair)][:, 0:512],
                    mybir.ActivationFunctionType.Identity,
                    bias=kq_bias("q", gpair),
                )
                nc.vector.tensor_scalar(
                    kq_dst("k", gpair, 0), pss[("k", gpair)][:, 0:512],
                    kq_bias("k", gpair), None, mybir.AluOpType.add,
                )
                kq_done[("k", gpair, 0)] = True
                kq_done[("q", gpair, 0)] = True

        emit_kq_startup()

        # ---- V units ----
        vaug = []

        def emit_v_unit(t):
            # cx banks are idle until ctx starts and have 2 bufs: V units
            # ping-pong there without the single-mm-bank WAR convoy
            ps = cx_ps.tile([128, C], F32, tag="cx", name=f"psv_{t}")
            for k in range(8):
                nc.tensor.matmul(
                    ps[:],
                    lhsT=xt[k][:, 128 * t : 128 * t + 128],
                    rhs=wv_sb[:, ts(k, C)],
                    start=(k == 0),
                    stop=(k == 7),
                )
            vt = vaug_pool.tile(
                [128, HPC * (HD + 1)], BF16, tag="vaug", name=f"vaug_{t}"
            )
            vt3 = vt[:].rearrange("p (h x) -> p h x", x=HD + 1)
            nc.vector.memset(vt3[:, :, 0:1], 1.0)
            nc.vector.tensor_copy(
                vt3[:, :, 1 : HD + 1],
                ps[:].rearrange("p (h d) -> p h d", d=HD),
            )
            vaug.append(vt)

        # ---- score chunks / exp groups ----
        # chunk c <-> (u, j, hh, tt); groups alternate scA (2 chunks) and
        # scB (3 chunks); the final group is a single scA chunk.
        UNITS = [(n, p) for n in range(4) for p in range(2)]
        N_CHUNK = 256

        def chunk_parts(c):
            u, r = divmod(c, 32)
            j, r = divmod(r, 4)
            hh, tt = divmod(r, 2)
            return UNITS[u], j, hh, tt

        groups = []
        c = 0
        while c < N_CHUNK:
            if len(groups) % 2 == 0:
                size = min(2, N_CHUNK - c)
            else:
                size = min(3, N_CHUNK - c)
            groups.append(list(range(c, c + size)))
            c += size

        chunk_loc = {}

        def emit_group(gi):
            chunks = groups[gi]
            wide = len(chunks) == 3
            cols = 512 * len(chunks)
            pool, tag, atp = ((scB_ps, "scB", atB_pool) if wide
                              else (scA_ps, "scA", atA_pool))
            sc = pool.tile([128, 1536 if wide else 1024], F32, tag=tag,
                           name=f"sc{gi}")
            at = atp.tile([128, 1536 if wide else 1024], BF16,
                          tag=("atB" if wide else "atA"), name=f"at{gi}")
            for i, ch in enumerate(chunks):
                (n, p), j, hh, tt = chunk_parts(ch)
                h = 2 * p + hh
                t = 2 * j + tt
                nc.tensor.matmul(
                    sc[:, ts(i, 512)],
                    lhsT=kt8r[32 * h : 32 * h + 32, :, 128 * t : 128 * t + 128],
                    rhs=qt8r[32 * h : 32 * h + 32, :, 512 * n : 512 * n + 512],
                    start=True, stop=True,
                    perf_mode=mybir.MatmulPerfMode.DoubleRow,
                    tile_position=(32 * h, 0),
                )
                chunk_loc[ch] = (at, 512 * i)
            nc.scalar.activation(
                at[:, 0:cols], sc[:, 0:cols],
                mybir.ActivationFunctionType.Exp, scale=0.125,
            )

        # ---- ctx accumulation / norm ----
        cx_tiles = {}
        ctxn_tiles = {}

        def emit_ctx(u, j):
            n, p = u
            for hh in range(2):
                h = 2 * p + hh
                key = (u, h)
                if key not in cx_tiles:
                    cx_tiles[key] = cx_ps.tile(
                        [128, 512], F32, tag="cx", name=f"cx_{n}_{h}"
                    )
                cx3 = cx_tiles[key][:, 0 : 4 * 65].rearrange(
                    "p (qt c) -> p qt c", c=65
                )
                for tt in range(2):
                    t = 2 * j + tt
                    ch = ((u[0] * 2 + u[1]) * 8 + j) * 4 + hh * 2 + tt
                    at, off = chunk_loc[ch]
                    for qt_i in range(4):
                        nc.tensor.matmul(
                            cx3[:, qt_i, :],
                            lhsT=at[:, off + 128 * qt_i : off + 128 * qt_i + 128],
                            rhs=vaug[t][:, 65 * h : 65 * h + 65],
                            start=(j == 0 and tt == 0 and qt_i == 0),
                            stop=(j == 7 and tt == 1 and qt_i == 3),
                            skip_group_check=True,
                        )

        def emit_norm(u, act_split=False):
            n, p = u
            for hh in range(2):
                h = 2 * p + hh
                cx = cx_tiles[(u, h)]
                cx3 = cx[:, 0 : 4 * 65].rearrange("p (qt c) -> p qt c", c=65)
                rc = recip_pool.tile([128, 4], F32, tag="rc", name=f"rc_{n}_{h}")
                nc.vector.reciprocal(rc[:], cx3[:, :, 0])
                for qt_i in range(4):
                    qg = 4 * n + qt_i
                    key = (qg, p)
                    if key not in ctxn_tiles:
                        ctxn_tiles[key] = ctxn_pool.tile(
                            [128, 128], BF16, tag="cn", name=f"cn_{qg}_{p}"
                        )
                    dst = ctxn_tiles[key][:, 64 * hh : 64 * hh + 64]
                    if act_split and hh == 1:
                        nc.scalar.mul(dst, cx3[:, qt_i, 1 : HD + 1],
                                      rc[:, qt_i : qt_i + 1])
                    else:
                        nc.vector.tensor_scalar(
                            dst, cx3[:, qt_i, 1 : HD + 1],
                            rc[:, qt_i : qt_i + 1], None,
                            mybir.AluOpType.mult,
                        )

        # ---- transpose + out projection ----
        ctxT_tiles = {}

        def emit_tr_unit(u, act_copies=False):
            n, p = u
            if (n, p) not in ctxT_tiles:
                ctxT_tiles[(n, p)] = ctxT_pool.tile(
                    [128, 512], BF16, tag="ct", name=f"ct_{n}_{p}"
                )
            cT = ctxT_tiles[(n, p)]
            trp = cx_ps.tile([128, 512], F32, tag="cx", name=f"tr_{n}_{p}")
            trb = trp[:].bitcast(BF16)
            for qt_i in range(4):
                qg = 4 * n + qt_i
                slot = trb[:, 256 * qt_i : 256 * qt_i + 128]
                nc.tensor.transpose(slot, ctxn_tiles[(qg, p)][:], ident[:])
                if act_copies and qt_i % 2 == 1:
                    nc.scalar.copy(cT[:, ts(qt_i, 128)], slot)
                else:
                    nc.vector.tensor_copy(cT[:, ts(qt_i, 128)], slot)

        ob_tiles = {}

        def emit_out_half(n, qt_i, dh, act_copies=False, split_dma=False,
                          drain_pools=None, drain_i=0):
            s = 4 * n + qt_i
            if s not in ob_tiles:
                ob_tiles[s] = outsb_pool.tile(
                    [128, D], BF16, tag="ob", name=f"ob_{s}"
                )
            ob = ob_tiles[s]
            if drain_pools is not None:
                pool, tag = drain_pools[drain_i % len(drain_pools)]
                ps0 = pool.tile([128, 1536 if tag == "scB" else
                                 1024 if tag == "scA" else 512],
                                F32, tag=tag, name=f"pso_{s}_{dh}")
                ps = ps0[:, 0:512]
            else:
                ps = mm_ps.tile([128, 512], F32, tag="mm",
                                name=f"pso_{s}_{dh}")[:]
            for p in range(2):
                nc.tensor.matmul(
                    ps,
                    lhsT=ctxT_tiles[(n, p)][:, ts(qt_i, 128)],
                    rhs=wo_sb[:, 1024 * p + 512 * dh : 1024 * p + 512 * dh + 512],
                    start=(p == 0),
                    stop=(p == 1),
                )
            if act_copies and dh == 1:
                nc.scalar.copy(ob[:, ts(dh, 512)], ps)
            else:
                nc.vector.tensor_copy(ob[:, ts(dh, 512)], ps)
            if split_dma:
                nc.sync.dma_start(
                    out[ts(s, 128), ts(dh, 512)], ob[:, ts(dh, 512)]
                )
            elif dh == 1:
                nc.sync.dma_start(out[ts(s, 128), :], ob[:])

        # ---- emission scheduler ----
        # ACT is the critical engine: keep a small lead of emitted exps and
        # backfill the PE with ctx/tr/v/out/kq quanta.  kq rounds that feed
        # upcoming groups are emitted deadline-first so a group never has to
        # force one synchronously (which would stall the exp stream).
        CHUNK = 107.0
        EXP = {1: 612.0, 2: 1038.0, 3: 1465.0}
        KQ_COST = 2100.0
        V_COST = 1250.0
        CTX_COST = 432.0
        TR_COST = 260.0
        OUT_COST = 700.0
        SLACK = 1500.0
        LAG_CHUNKS = 12

        st = {"pe": 12000.0, "act": 12200.0, "gi": 0, "ci": 0, "vi": 0,
              "chunks": 0}
        exp_end = {}
        norm_done = {}
        # (which, g, n) -> first group index that needs the slice
        kq_need = {}
        for n_, gidx in ((1, 3), (2, 6), (3, 9)):
            kq_need[("k", 0, n_)] = gidx
            kq_need[("k", 1, n_)] = gidx
        for n_, gidx in ((1, 25), (2, 51), (3, 76)):
            kq_need[("q", 0, n_)] = gidx
            kq_need[("q", 1, n_)] = gidx
        kq_queue = sorted(kq_need, key=lambda k: kq_need[k])
        tr_queue = []
        out_queue = []
        ctx_stream = [(u, j) for u in UNITS for j in range(8)]

        def kq_arrival(key):
            return t_xq[key[2]]

        def v_arrival(t):
            return max(t_wv, t_xq[t // 4])

        def do_kq(key):
            if kq_done.get(key):
                return
            which, g, n = key
            # cx banks are idle until the deferred V units start (group ~15):
            # ping-pong the early kt rounds there instead of serializing
            # them on the single mm bank; late q rounds stay on mm (cx is
            # owned by ctx accumulators by then)
            if which == "k":
                emit_kq_round(which, g, n, cx_ps, "cx")
            else:
                emit_kq_round(which, g, n, mm_ps, "mm")
            if key in kq_queue:
                kq_queue.remove(key)
            st["pe"] = max(st["pe"], kq_arrival(key)) + KQ_COST

        def do_group():
            gi = st["gi"]
            for ch in groups[gi]:
                (n, p), j, hh, tt = chunk_parts(ch)
                t = 2 * j + tt
                for key in (("q", 0, n), ("q", 1, n),
                            ("k", 0, t // 4), ("k", 1, t // 4)):
                    do_kq(key)
            emit_group(gi)
            # piggyback at most one V unit per group: spacing them out lets
            # the score chunks absorb each unit's vaug-copy WAR wait instead
            # of forming a serialized convoy on the single mm psum bank
            vtarget = min(16, 2 * max(0, gi - 22))
            for _ in range(2):
                if (st["vi"] < vtarget
                        and st["pe"] + 400.0 >= v_arrival(st["vi"])):
                    emit_v_unit(st["vi"])
                    st["pe"] = max(st["pe"], v_arrival(st["vi"])) + V_COST
                    st["vi"] += 1
            st["pe"] += len(groups[gi]) * CHUNK
            if gi >= 2:
                st["pe"] = max(st["pe"], exp_end.get(gi - 2, 0.0))
            s0 = max(st["act"], st["pe"] + 100.0)
            e = s0 + EXP[len(groups[gi])]
            exp_end[gi] = e
            st["act"] = e
            st["chunks"] += len(groups[gi])
            st["gi"] += 1

        def ctx_ready(i, horizon=0.0):
            if i >= len(ctx_stream):
                return False
            u, j = ctx_stream[i]
            last_ch = ((u[0] * 2 + u[1]) * 8 + j) * 4 + 3
            if last_ch not in chunk_loc:
                return False
            if st["chunks"] - last_ch < LAG_CHUNKS and st["gi"] < len(groups):
                return False
            if st["vi"] < 16:
                # V units live in the cx banks: every V psum must be
                # allocated (and thus emitted) before the first ctx tile
                return False
            gi_of = [g for g in exp_end if groups[g][-1] >= last_ch >= groups[g][0]]
            e = exp_end.get(gi_of[0], 0.0) if gi_of else 0.0
            return e <= st["pe"] + horizon

        def do_ctx():
            u, j = ctx_stream[st["ci"]]
            emit_ctx(u, j)
            st["pe"] += CTX_COST
            if j == 7:
                emit_norm(u, act_split=(st["ci"] == len(ctx_stream) - 1))
                norm_done[u] = st["pe"] + 1400.0
                tr_queue.append(u)
            st["ci"] += 1

        def kq_urgent():
            if not kq_queue:
                return None
            key = kq_queue[0]
            # kt rounds (need<=9) go as soon as possible; q rounds wait out
            # the congested early window but spread over 24 groups
            win = 24 if kq_need[key] >= 20 and st["gi"] >= 16 else 10
            if kq_need[key] > st["gi"] + win:
                return None
            if (st["pe"] + 1200.0 >= kq_arrival(key)
                    or kq_need[key] <= st["gi"] + 3):
                return key
            return None

        def filler(budget, act_copies=False, drain_pools=None, di=[0]):
            key = kq_urgent()
            if key is not None and budget + 1500.0 >= KQ_COST:
                do_kq(key)
                return True
            if tr_queue and budget >= TR_COST:
                u = tr_queue.pop(0)
                emit_tr_unit(u, act_copies=act_copies)
                st["pe"] = max(st["pe"] + TR_COST, norm_done.get(u, 0.0))
                if u[1] == 1:
                    for qt_i in range(4):
                        out_queue.append((u[0], qt_i, 0))
                        out_queue.append((u[0], qt_i, 1))
                return True
            if (st["ci"] < len(ctx_stream) and budget >= CTX_COST
                    and ctx_ready(st["ci"], horizon=200.0)):
                do_ctx()
                return True
            if out_queue and budget >= OUT_COST:
                n, qt_i, dh = out_queue.pop(0)
                emit_out_half(n, qt_i, dh, act_copies=act_copies,
                              split_dma=act_copies, drain_pools=drain_pools,
                              drain_i=di[0])
                di[0] += 1
                st["pe"] += OUT_COST
                return True
            if kq_queue and budget >= KQ_COST:
                do_kq(kq_queue[0])
                return True
            if st["vi"] < 16 and budget >= 2 * V_COST:
                # force V even if the DMA estimate says we'd stall
                emit_v_unit(st["vi"])
                st["pe"] = max(st["pe"], v_arrival(st["vi"])) + V_COST
                st["vi"] += 1
                return True
            return False

        # at pools hold 8 tiles (~20 chunks each kind): never let the score
        # stream get more than MAX_LAG chunks ahead of ctx consumption or
        # the at-slot WAR chain wraps around into a queue deadlock
        MAX_LAG = 80
        guard = 0
        while st["gi"] < len(groups):
            guard += 1
            assert guard < 30000, "emission scheduler runaway"
            lag = st["chunks"] - 4 * st["ci"]
            if lag > MAX_LAG and st["ci"] < len(ctx_stream):
                if tr_queue:
                    u = tr_queue.pop(0)
                    emit_tr_unit(u)
                    st["pe"] = max(st["pe"] + TR_COST, norm_done.get(u, 0.0))
                    if u[1] == 1:
                        for qt_i in range(4):
                            out_queue.append((u[0], qt_i, 0))
                            out_queue.append((u[0], qt_i, 1))
                    continue
                while st["vi"] < 16:
                    emit_v_unit(st["vi"])
                    st["pe"] = max(st["pe"], v_arrival(st["vi"])) + V_COST
                    st["vi"] += 1
                do_ctx()
                continue
            # free PE window: group gi's chunks cannot start before
            # exp_end[gi-2] (scores-tile WAR), so filler work up to that
            # point is free; urgent kq/v may overrun by one exp of ACT lead
            budget = exp_end.get(st["gi"] - 2, 12200.0) - st["pe"]
            if budget <= 0 or not filler(budget):
                do_group()
            import os
            if os.environ.get("KDBG"):
                print(f"gi={st['gi']:3d} ci={st['ci']:3d} vi={st['vi']:2d} "
                      f"lag={st['chunks']-4*st['ci']:3d} pe={st['pe']:8.0f} "
                      f"act={st['act']:8.0f} outq={len(out_queue)} "
                      f"kqq={len(kq_queue)}")

        # drain phase: everything left, exp stream done -> ACT helps copy,
        # out psums borrow the dead scores banks
        drain_pools = [(scA_ps, "scA"), (scB_ps, "scB"), (mm_ps, "mm")]
        guard = 0
        while (st["ci"] < len(ctx_stream) or tr_queue or out_queue
               or kq_queue or st["vi"] < 16):
            guard += 1
            assert guard < 30000, "drain runaway"
            if st["vi"] < 16:
                emit_v_unit(st["vi"])
                st["vi"] += 1
                continue
            if tr_queue:
                u = tr_queue.pop(0)
                emit_tr_unit(u, act_copies=True)
                if u[1] == 1:
                    for qt_i in range(4):
                        out_queue.append((u[0], qt_i, 0))
                        out_queue.append((u[0], qt_i, 1))
                continue
            if st["ci"] < len(ctx_stream):
                do_ctx()
                continue
            assert filler(1e18, act_copies=True, drain_pools=drain_pools)

    nc.compile()
    return nc


def _get_nc():
    if "nc" not in _CACHE:
        _CACHE["nc"] = _build_nc()
    return _CACHE["nc"]


def _make_in_maps(inputs):
    x = np.asarray(inputs["x"], np.float32)
    Wq = np.asarray(inputs["Wq"], np.float32)
    Wk = np.asarray(inputs["Wk"], np.float32)
    Wv = np.asarray(inputs["Wv"], np.float32)
    Wo = np.asarray(inputs["Wo"], np.float32)
    bq = np.asarray(inputs["bq"], np.float32)
    bk = np.asarray(inputs["bk"], np.float32)

    # column order within one 128-wide d-half group: j -> head j//32,
    # d = 32*g + j%32
    j = np.arange(128)
    col = {g: (j // 32) * 64 + 32 * g + (j % 32) for g in range(2)}

    def tile_w(w_slice):
        # [1024, 256] -> [2, 128, 8*128]: g-half, partition (x dim within
        # k-tile), k-tile-major columns
        o = np.empty((2, 128, 8 * 128), np.float32)
        for g in range(2):
            wg = w_slice[:, col[g]]          # [1024, 128]
            o[g] = wg.reshape(8, 128, 128).transpose(1, 0, 2).reshape(128, 8 * 128)
        return np.ascontiguousarray(o).astype(NP_BF16)

    def tile_b(b_slice):
        o = np.empty((128, 2), np.float32)
        for g in range(2):
            o[:, g] = b_slice[col[g]]
        return np.ascontiguousarray(o)

    def tile_wv(w_slice):
        return np.ascontiguousarray(
            w_slice.reshape(8, 128, C).transpose(1, 0, 2).reshape(128, 8 * C)
        ).astype(NP_BF16)

    in_maps = []
    for core in range(N_CORES):
        b, hg = core // 4, core % 4
        cs = slice(C * hg, C * (hg + 1))
        xTc = np.ascontiguousarray(x[b].T).astype(NP_BF16)
        wo_rows = Wo[cs]  # [256, D]
        wo_c = np.concatenate([wo_rows[0:128], wo_rows[128:256]], axis=1)
        in_maps.append(
            {
                "xT": xTc,
                "wq": tile_w(Wq[:, cs]),
                "wk": tile_w(Wk[:, cs]),
                "wv": tile_wv(Wv[:, cs]),
                "wo": np.ascontiguousarray(wo_c).astype(NP_BF16),
                "bq": tile_b(bq[cs]),
                "bk": tile_b(bk[cs]),
            }
        )
    return in_maps


def run(inputs, trace=False):
    """Run the SPMD kernel; returns (full_output, BassKernelResults)."""
    nc = _get_nc()
    in_maps = _make_in_maps(inputs)
    res = bass_utils.run_bass_kernel_spmd(
        nc, in_maps, core_ids=list(range(N_CORES)), trace=trace
    )
    bo = np.asarray(inputs["bo"], np.float32)
    bv = np.asarray(inputs["bv"], np.float32)
    Wo = np.asarray(inputs["Wo"], np.float32)
    bo_aug = bo + bv @ Wo
    full = np.empty((2, S, D), np.float32)
    for b in range(2):
        acc = res.results[4 * b]["out"].astype(np.float32).copy()
        for hg in range(1, 4):
            acc += res.results[4 * b + hg]["out"]
        full[b] = acc + bo_aug
    return full, res


def kernel(**inputs):
    full, _ = run(inputs, trace=False)
    return full
